# revision 17
# baseline (speedup 1.0000x reference)
"""DifferentialAttention Trainium2 Bass kernel.

Sharding: 8 cores = 2 batches x 4 head-pairs (2 heads each).
Per core (SPMD, same program, different data):
  phase A: q/k/v projections (bf16 matmuls, f32 PSUM) + RMS-norm + per-head
           "rotary" (constant per head due to reference's head-axis positions)
           + PE transpose into [d, t] layout.
  phase B: differential causal attention per head, exp without max-subtract
           (|S*scale| <= sqrt(128), safe in f32), softmax denominator via
           gpsimd accumulate + ones-matmul partition reduce, normalization
           applied to O^T after a gpsimd partition broadcast of 1/den.
  phase C: output projection (0.8 prescale folded into woT host-side);
           per-core partial y summed over head-pair cores on host.
"""

import json
import os
import sys
import tempfile
from contextlib import ExitStack

import numpy as np

sys.path.insert(0, "/opt/trn_rl_repo")

import ml_dtypes  # noqa: E402

import concourse.bass as bass  # noqa: E402
import concourse.mybir as mybir  # noqa: E402
import concourse.tile as tile  # noqa: E402
from concourse import bacc, bass_utils  # noqa: E402
from concourse.masks import make_identity  # noqa: E402

B, T, C = 2, 2048, 2048
NH, HD, HH = 8, 256, 128
LAMBDA_INIT = 0.2
RMS_EPS = 1.1920929e-07
SCALE = float(1.0 / np.sqrt(np.float32(HH)))

F32 = mybir.dt.float32
BF16 = mybir.dt.bfloat16
NPBF16 = ml_dtypes.bfloat16

NM = T // 128          # 16 m-tiles (t blocks)
NK = C // 128          # 16 k-tiles (c blocks)
NCH = T // 512         # 4 tq chunks

_ACT_TABLES_DONE = False


def _setup_act_tables():
    """Reorder act_info so `natural_log_exp_and_others` is the first table:
    it covers every ACT func we use (square, ln, exp, copy), so the greedy
    table selector stays on one table instead of thrashing exp<->ln loads."""
    global _ACT_TABLES_DONE
    if _ACT_TABLES_DONE:
        return
    from neuronxcc.driver.Job import Job  # noqa: PLC0415
    from neuronxcc.driver.jobs.support.FindActInfo import (  # noqa: PLC0415
        findActInfoFile,
    )

    src = findActInfoFile(Job.getPackageDir(), "gen3")
    srcdir = os.path.dirname(src)
    with open(src) as f:
        info = json.load(f)
    info["act_func_sets"].sort(
        key=lambda s: s["name"] != "natural_log_exp_and_others")
    dstdir = os.path.join(tempfile.gettempdir(), "act_info_nlexp_first")
    os.makedirs(dstdir, exist_ok=True)
    for name in os.listdir(srcdir):
        dst = os.path.join(dstdir, name)
        if not os.path.exists(dst):
            try:
                os.symlink(os.path.join(srcdir, name), dst)
            except OSError:
                pass
    act_path = os.path.join(dstdir, "act_info.json")
    with open(act_path, "w") as f:
        json.dump(info, f)
    os.environ["BASS_ACT_ROOT_JSON_PATH"] = act_path

    import concourse.hw_specs as hw_specs  # noqa: PLC0415

    def patched(module_arch):
        return {
            e["name"]: {
                mybir.ActivationFunctionType.from_pwp(v) for v in e["act"]
            }
            for e in info["act_func_sets"]
        }

    hw_specs.get_activation_tables = patched
    bacc.get_activation_tables = patched
    _ACT_TABLES_DONE = True


def _bcast_cols(ap2d, col0, nblk, inner):
    """[128, nblk, inner] view of columns col0..col0+nblk of a [128, n] tile,
    each column replicated `inner` times along a 0-stride inner dim."""
    return bass.AP(
        tensor=ap2d.tensor,
        offset=ap2d.offset + col0,
        ap=[ap2d.ap[0], [1, nblk], [0, inner]],
    )


def _body(tc, aps):
    nc = tc.nc
    xT, wqT, wkT, wvT, woT, trig, tri, ones, neglam, y = aps

    xT_r = xT.rearrange("(k p) t -> p k t", p=128)      # [128, 16, 2048]
    wqT_r = wqT.rearrange("(k p) n -> p k n", p=128)    # [128, 16, 512]
    wkT_r = wkT.rearrange("(k p) n -> p k n", p=128)
    wvT_r = wvT.rearrange("(k p) n -> p k n", p=128)
    woT_r = woT.rearrange("(k p) n -> p k n", p=128)    # [128, 4, 2048]

    with ExitStack() as octx:
        # ---- persistent tiles (live across phases) ----
        persist = octx.enter_context(tc.tile_pool(name="persist", bufs=1))
        qkT_all = persist.tile([128, 8, T], BF16)    # seg h*4 + [q1,q2,k1,k2]
        v_all = persist.tile([128, NM, 512], BF16)   # [t(P) per m, e: h0|h1]
        trig_sb = persist.tile([128, 512], F32)      # cosE | sinE, block-major
        tri_sb = persist.tile([128, 128], BF16)
        ones_sb = persist.tile([128, 1], F32)
        neglam_sb = persist.tile([128, 1], F32)
        ident_sb = persist.tile([128, 128], BF16)

        nc.sync.dma_start(out=trig_sb, in_=trig)
        nc.sync.dma_start(out=tri_sb, in_=tri)
        nc.sync.dma_start(out=ones_sb, in_=ones)
        nc.sync.dma_start(out=neglam_sb, in_=neglam)
        make_identity(nc, ident_sb)

        cosE = trig_sb[:, 0:256].rearrange("p (b i) -> p b i", b=4)
        sinE = trig_sb[:, 256:512].rearrange("p (b i) -> p b i", b=4)

        # segment mapping: qkT_all viewed [128, h, 4, T]; q -> [:, :, 0:2],
        # k -> [:, :, 2:4]; block order within a group is (h0b1,h0b2,h1b1,h1b2)
        qkT_v = qkT_all.rearrange("p (h f) t -> p h f t", h=2)

        # ================= Phase A: projections =================
        with ExitStack() as actx:
            wpool = actx.enter_context(tc.tile_pool(name="wpool", bufs=1))
            xpool = actx.enter_context(tc.tile_pool(name="xpool", bufs=1))
            psA = actx.enter_context(
                tc.tile_pool(name="psA", bufs=6, space="PSUM"))
            psT = actx.enter_context(
                tc.tile_pool(name="psT", bufs=2, space="PSUM"))
            sbA = actx.enter_context(tc.tile_pool(name="sbA", bufs=2))

            wq_sb = wpool.tile([128, NK, 512], BF16)
            wk_sb = wpool.tile([128, NK, 512], BF16)
            wv_sb = wpool.tile([128, NK, 512], BF16)
            x_sb = xpool.tile([128, NK, T], BF16)
            for kk in range(NK):
                nc.sync.dma_start(out=x_sb[:, kk, :], in_=xT_r[:, kk, :])
                nc.sync.dma_start(out=wq_sb[:, kk, :], in_=wqT_r[:, kk, :])
                nc.sync.dma_start(out=wk_sb[:, kk, :], in_=wkT_r[:, kk, :])
                nc.sync.dma_start(out=wv_sb[:, kk, :], in_=wvT_r[:, kk, :])

            for m in range(NM):
                mlo, mhi = m * 128, (m + 1) * 128
                q_ps = psA.tile([128, 512], F32, tag="acc")
                k_ps = psA.tile([128, 512], F32, tag="acc")
                v_ps = psA.tile([128, 512], F32, tag="acc")
                for kk in range(NK):
                    lhs = x_sb[:, kk, mlo:mhi]
                    st, sp = kk == 0, kk == NK - 1
                    nc.tensor.matmul(q_ps, lhs, wq_sb[:, kk, :], start=st, stop=sp)
                    nc.tensor.matmul(k_ps, lhs, wk_sb[:, kk, :], start=st, stop=sp)
                    nc.tensor.matmul(v_ps, lhs, wv_sb[:, kk, :], start=st, stop=sp)

                # vacate PSUM quickly: stage q/k to SBUF, v straight out
                qs = sbA.tile([128, 512], F32, tag="qs")
                nc.scalar.copy(qs, q_ps)
                ks = sbA.tile([128, 512], F32, tag="ks")
                nc.vector.tensor_copy(ks, k_ps)
                nc.scalar.copy(v_all[:, m, :], v_ps)

                # sumsq of the 8 blocks: stats cols 0-3 = q, 4-7 = k
                stats = sbA.tile([128, 8], F32, tag="stats")
                sq_scr = sbA.tile([128, 128], F32, tag="sqscr")
                for j in range(8):
                    src = qs if j < 4 else ks
                    off = (j % 4) * 128
                    nc.scalar.activation(
                        sq_scr, src[:, off:off + 128],
                        mybir.ActivationFunctionType.Square,
                        accum_out=stats[:, j:j + 1])
                # rstd = exp(-0.5*ln(mean+eps)) -- ln/exp/square share a table
                veps = sbA.tile([128, 8], F32, tag="veps")
                nc.vector.tensor_scalar(
                    veps, stats, 1.0 / HH, RMS_EPS,
                    mybir.AluOpType.mult, mybir.AluOpType.add)
                lnv = sbA.tile([128, 8], F32, tag="lnv")
                nc.scalar.activation(
                    lnv, veps, mybir.ActivationFunctionType.Ln)
                rstd8 = sbA.tile([128, 8], F32, tag="rstd8")
                nc.scalar.activation(
                    rstd8, lnv, mybir.ActivationFunctionType.Exp, scale=-0.5)

                # normalize + rotary, batched over all 4 blocks of q then k
                for g, (src, c0) in enumerate(((qs, 0), (ks, 4))):
                    qn = sbA.tile([128, 512], F32, tag="qn")
                    nc.vector.tensor_tensor(
                        qn.rearrange("p (b i) -> p b i", b=4), src.rearrange(
                            "p (b i) -> p b i", b=4),
                        _bcast_cols(rstd8, c0, 4, 128), mybir.AluOpType.mult)
                    qn3 = qn.rearrange("p (b i) -> p b i", b=4)
                    qn_lo, qn_hi = qn3[:, :, 0:64], qn3[:, :, 64:128]
                    t1 = sbA.tile([128, 256], F32, tag="t1")
                    t13 = t1.rearrange("p (b i) -> p b i", b=4)
                    t2 = sbA.tile([128, 256], F32, tag="t2")
                    t23 = t2.rearrange("p (b i) -> p b i", b=4)
                    qnr = sbA.tile([128, 512], BF16, tag="qnr")
                    qnr3 = qnr.rearrange("p (b i) -> p b i", b=4)
                    nc.vector.tensor_tensor(t13, qn_lo, cosE,
                                            mybir.AluOpType.mult)
                    nc.vector.tensor_tensor(t23, qn_hi, sinE,
                                            mybir.AluOpType.mult)
                    nc.vector.tensor_add(qnr3[:, :, 0:64], t13, t23)
                    nc.vector.tensor_tensor(t13, qn_hi, cosE,
                                            mybir.AluOpType.mult)
                    nc.vector.tensor_tensor(t23, qn_lo, sinE,
                                            mybir.AluOpType.mult)
                    nc.vector.tensor_sub(qnr3[:, :, 64:128], t13, t23)

                    tp = psT.tile([128, 512], BF16, tag="tp")
                    for bi in range(4):
                        nc.tensor.transpose(
                            tp[:, bi * 128:(bi + 1) * 128],
                            qnr[:, bi * 128:(bi + 1) * 128], ident_sb)
                    # dst: [128, h(2), br(2), 128] at segment group g
                    nc.vector.tensor_copy(
                        qkT_v[:, :, 2 * g:2 * g + 2, mlo:mhi],
                        tp.rearrange("p (h b i) -> p h b i", h=2, b=2))

        # ================= Phase B: attention =================
        bcpool = octx.enter_context(tc.tile_pool(name="bcpool", bufs=1))
        oT_all = bcpool.tile([128, 4, T], BF16)      # seg h*2 + etile
        wo_sb = bcpool.tile([128, 4, T], BF16)
        nc.sync.dma_start(out=wo_sb[:, 0, :], in_=woT_r[:, 0, :])
        nc.sync.dma_start(out=wo_sb[:, 1, :], in_=woT_r[:, 1, :])
        nc.sync.dma_start(out=wo_sb[:, 2, :], in_=woT_r[:, 2, :])
        nc.sync.dma_start(out=wo_sb[:, 3, :], in_=woT_r[:, 3, :])

        with ExitStack() as bctx:
            psS = bctx.enter_context(
                tc.tile_pool(name="psS", bufs=3, space="PSUM"))
            psO = bctx.enter_context(
                tc.tile_pool(name="psO", bufs=4, space="PSUM"))
            sbPT = bctx.enter_context(tc.tile_pool(name="sbPT", bufs=34))
            sbB = bctx.enter_context(tc.tile_pool(name="sbB", bufs=4))

            for h in range(2):
                qT = [qkT_v[:, h, 0, :], qkT_v[:, h, 1, :]]
                kT = [qkT_v[:, h, 2, :], qkT_v[:, h, 3, :]]
                for ch in range(NCH):
                    c0, c1 = ch * 512, ch * 512 + 512
                    ntk = 4 * ch + 4
                    # --- S + exp + den-accumulate, both branches ---
                    br_pts, br_acc = [], []
                    for br in range(2):
                        acc = sbB.tile([128, 512], F32, tag="acc")
                        pts = []
                        for tkb in range(ntk):
                            n0 = max(c0, tkb * 128)
                            nN = c1 - n0
                            col0 = n0 - c0
                            s_ps = psS.tile([128, 512], F32, tag="s")
                            nc.tensor.matmul(
                                s_ps[:, :nN],
                                kT[br][:, tkb * 128:(tkb + 1) * 128],
                                qT[br][:, n0:c1], start=True, stop=True)
                            pt = sbPT.tile([128, 512], BF16, tag="pt")
                            nc.scalar.activation(
                                pt[:, :nN], s_ps[:, :nN],
                                mybir.ActivationFunctionType.Exp, scale=SCALE)
                            if tkb * 128 >= c0:   # diagonal block
                                nc.vector.tensor_mul(
                                    pt[:, :128], pt[:, :128], tri_sb)
                            if tkb == 0:
                                nc.gpsimd.tensor_copy(acc, pt)
                            else:
                                nc.gpsimd.tensor_tensor(
                                    acc[:, col0:], acc[:, col0:], pt[:, :nN],
                                    mybir.AluOpType.add)
                            pts.append((tkb, pt, nN, col0))
                        br_pts.append(pts)
                        br_acc.append(acc)
                    # --- PV + den reduce, PE order: PV0, den0, PV1, den1 ---
                    o_ps = {}
                    den_ps = {}
                    for br in range(2):
                        o_ps[br] = [psO.tile([128, 512], F32, tag="o",
                                             name=f"o_ps{br}{e}")
                                    for e in range(2)]
                        for e in range(2):
                            ecol = h * 256 + e * 128
                            for i, (tkb, pt, nN, col0) in enumerate(br_pts[br]):
                                nc.tensor.matmul(
                                    o_ps[br][e][:, col0:],
                                    v_all[:, tkb, ecol:ecol + 128],
                                    pt[:, :nN],
                                    start=(i == 0), stop=(i == ntk - 1))
                        dp = psS.tile([1, 512], F32, tag="s", name=f"den{br}")
                        nc.tensor.matmul(dp, ones_sb, br_acc[br],
                                         start=True, stop=True)
                        den_ps[br] = dp
                    # --- 1/den (fast approx), broadcast, combine ---
                    invb = {}
                    for br in range(2):
                        inv = sbB.tile([1, 512], F32, tag="inv",
                                       name=f"inv{br}")
                        nc.vector.reciprocal_approx_fast(inv, den_ps[br])
                        ib = sbB.tile([128, 512], F32, tag="invb",
                                      name=f"invb{br}")
                        nc.gpsimd.partition_broadcast(ib, inv)
                        invb[br] = ib
                    for e in range(2):
                        o1n = sbB.tile([128, 512], F32, tag="o1n")
                        o2n = sbB.tile([128, 512], F32, tag="o2n")
                        nc.vector.tensor_mul(o1n, o_ps[0][e], invb[0])
                        nc.vector.tensor_mul(o2n, o_ps[1][e], invb[1])
                        nc.vector.scalar_tensor_tensor(
                            oT_all[:, h * 2 + e, c0:c1], o2n, neglam_sb, o1n,
                            mybir.AluOpType.mult, mybir.AluOpType.add)

        # ================= Phase C: out projection =================
        with ExitStack() as cctx:
            psY = cctx.enter_context(
                tc.tile_pool(name="psY", bufs=8, space="PSUM"))
            sbY = cctx.enter_context(tc.tile_pool(name="sbY", bufs=2))
            for m in range(NM):
                mlo, mhi = m * 128, (m + 1) * 128
                y_ps = [psY.tile([128, 512], F32, tag="y", name=f"y_ps{cc}")
                        for cc in range(4)]
                for kk in range(4):
                    lhs = oT_all[:, kk, mlo:mhi]
                    for cc in range(4):
                        nc.tensor.matmul(
                            y_ps[cc], lhs, wo_sb[:, kk, cc * 512:(cc + 1) * 512],
                            start=(kk == 0), stop=(kk == 3))
                ystage = sbY.tile([128, T], F32, tag="ystage")
                for cc in range(4):
                    if cc % 2 == 0:
                        nc.vector.tensor_copy(
                            ystage[:, cc * 512:(cc + 1) * 512], y_ps[cc])
                    else:
                        nc.scalar.copy(
                            ystage[:, cc * 512:(cc + 1) * 512], y_ps[cc])
                nc.sync.dma_start(out=y[mlo:mhi, :], in_=ystage)


def build_nc():
    _setup_act_tables()
    nc = bacc.Bacc("TRN2", target_bir_lowering=False, debug=False,
                   num_devices=8)
    xT = nc.dram_tensor("xT", [C, T], BF16, kind="ExternalInput").ap()
    wqT = nc.dram_tensor("wqT", [C, 512], BF16, kind="ExternalInput").ap()
    wkT = nc.dram_tensor("wkT", [C, 512], BF16, kind="ExternalInput").ap()
    wvT = nc.dram_tensor("wvT", [C, 512], BF16, kind="ExternalInput").ap()
    woT = nc.dram_tensor("woT", [512, C], BF16, kind="ExternalInput").ap()
    trig = nc.dram_tensor("trig", [128, 512], F32, kind="ExternalInput").ap()
    tri = nc.dram_tensor("tri", [128, 128], BF16, kind="ExternalInput").ap()
    ones = nc.dram_tensor("ones", [128, 1], F32, kind="ExternalInput").ap()
    neglam = nc.dram_tensor("neglam", [128, 1], F32,
                            kind="ExternalInput").ap()
    y = nc.dram_tensor("y", [T, C], F32, kind="ExternalOutput").ap()
    with tile.TileContext(nc) as tc:
        _body(tc, (xT, wqT, wkT, wvT, woT, trig, tri, ones, neglam, y))
    nc.compile()
    return nc


def _host_prep(x, wq, wk, wv, wo, lq1, lk1, lq2, lk2):
    x = np.asarray(x, np.float32)
    wq, wk, wv, wo = (np.asarray(w, np.float32) for w in (wq, wk, wv, wo))
    lam = float(np.exp(np.sum(np.asarray(lq1, np.float32) *
                              np.asarray(lk1, np.float32))) -
                np.exp(np.sum(np.asarray(lq2, np.float32) *
                              np.asarray(lk2, np.float32))) + LAMBDA_INIT)

    d = HH
    inv_freq = (1.0 / 10000.0) ** (np.arange(0, d, 2, dtype=np.float32) / d)
    freqs = np.outer(np.arange(NH, dtype=np.float32), inv_freq)
    cos, sin = np.cos(freqs), np.sin(freqs)

    tri = np.triu(np.ones((128, 128), np.float32)).astype(NPBF16)
    ones = np.ones((128, 1), np.float32)
    neglam = np.full((128, 1), -lam, np.float32)

    in_maps = []
    for core in range(8):
        b = core // 4
        hp = core % 4
        h0, h1 = 2 * hp, 2 * hp + 1
        rows = np.r_[h0 * 256:(h0 + 1) * 256, h1 * 256:(h1 + 1) * 256]
        # block order (h0, h0, h1, h1): cosE | sinE, each [128, 4*64]
        cosE = np.concatenate(
            [np.tile(cos[hh][None, :], (128, 1)) for hh in (h0, h0, h1, h1)],
            axis=1)
        sinE = np.concatenate(
            [np.tile(sin[hh][None, :], (128, 1)) for hh in (h0, h0, h1, h1)],
            axis=1)
        trig_t = np.concatenate([cosE, sinE], axis=1).astype(np.float32)
        in_maps.append({
            "xT": np.ascontiguousarray(x[b].T).astype(NPBF16),
            "wqT": np.ascontiguousarray(wq[rows, :].T).astype(NPBF16),
            "wkT": np.ascontiguousarray(wk[rows, :].T).astype(NPBF16),
            "wvT": np.ascontiguousarray(wv[rows, :].T).astype(NPBF16),
            "woT": np.ascontiguousarray(
                (wo[:, rows].T * (1.0 - LAMBDA_INIT))).astype(NPBF16),
            "trig": trig_t,
            "tri": tri,
            "ones": ones,
            "neglam": neglam,
        })
    return in_maps


def kernel(x, wq, wk, wv, wo, lq1, lk1, lq2, lk2, _results_out=None,
           _trace=False):
    in_maps = _host_prep(x, wq, wk, wv, wo, lq1, lk1, lq2, lk2)
    nc = build_nc()
    res = bass_utils.run_bass_kernel_spmd(nc, in_maps,
                                          core_ids=list(range(8)),
                                          trace=_trace)
    if _results_out is not None:
        _results_out.append(res)
    out = np.zeros((B, T, C), np.float32)
    for core in range(8):
        out[core // 4] += res.results[core]["y"]
    return out


# revision 20
# speedup vs baseline: 1.3997x; 1.3997x over previous
"""DifferentialAttention Trainium2 Bass kernel.

Sharding: 8 cores = 2 batches x 4 head-pairs (2 heads each).
Per core (SPMD, same program, different data):
  phase A: q/k/v projections (bf16 matmuls, f32 PSUM) + RMS-norm + per-head
           "rotary" (constant per head due to reference's head-axis positions)
           + PE transpose into [d, t] layout.
  phase B: differential causal attention per head, exp without max-subtract
           (|S*scale| <= sqrt(128), safe in f32), softmax denominator via
           gpsimd accumulate + ones-matmul partition reduce, normalization
           applied to O^T after a gpsimd partition broadcast of 1/den.
  phase C: output projection (0.8 prescale folded into woT host-side);
           per-core partial y summed over head-pair cores on host.
"""

import json
import os
import sys
import tempfile
from contextlib import ExitStack

import numpy as np

sys.path.insert(0, "/opt/trn_rl_repo")

import ml_dtypes  # noqa: E402

import concourse.bass as bass  # noqa: E402
import concourse.mybir as mybir  # noqa: E402
import concourse.tile as tile  # noqa: E402
from concourse import bacc, bass_utils  # noqa: E402
from concourse.masks import make_identity  # noqa: E402

B, T, C = 2, 2048, 2048
NH, HD, HH = 8, 256, 128
LAMBDA_INIT = 0.2
RMS_EPS = 1.1920929e-07
SCALE = float(1.0 / np.sqrt(np.float32(HH)))

F32 = mybir.dt.float32
BF16 = mybir.dt.bfloat16
NPBF16 = ml_dtypes.bfloat16

NM = T // 128          # 16 m-tiles (t blocks)
NK = C // 128          # 16 k-tiles (c blocks)
NCH = T // 512         # 4 tq chunks

_ACT_TABLES_DONE = False


def _setup_act_tables():
    """Reorder act_info so `natural_log_exp_and_others` is the first table:
    it covers every ACT func we use (square, ln, exp, copy), so the greedy
    table selector stays on one table instead of thrashing exp<->ln loads."""
    global _ACT_TABLES_DONE
    if _ACT_TABLES_DONE:
        return
    from neuronxcc.driver.Job import Job  # noqa: PLC0415
    from neuronxcc.driver.jobs.support.FindActInfo import (  # noqa: PLC0415
        findActInfoFile,
    )

    src = findActInfoFile(Job.getPackageDir(), "gen3")
    srcdir = os.path.dirname(src)
    with open(src) as f:
        info = json.load(f)
    info["act_func_sets"].sort(
        key=lambda s: s["name"] != "natural_log_exp_and_others")
    dstdir = os.path.join(tempfile.gettempdir(), "act_info_nlexp_first")
    os.makedirs(dstdir, exist_ok=True)
    for name in os.listdir(srcdir):
        dst = os.path.join(dstdir, name)
        if not os.path.exists(dst):
            try:
                os.symlink(os.path.join(srcdir, name), dst)
            except OSError:
                pass
    act_path = os.path.join(dstdir, "act_info.json")
    with open(act_path, "w") as f:
        json.dump(info, f)
    os.environ["BASS_ACT_ROOT_JSON_PATH"] = act_path

    import concourse.hw_specs as hw_specs  # noqa: PLC0415

    def patched(module_arch):
        return {
            e["name"]: {
                mybir.ActivationFunctionType.from_pwp(v) for v in e["act"]
            }
            for e in info["act_func_sets"]
        }

    hw_specs.get_activation_tables = patched
    bacc.get_activation_tables = patched
    _ACT_TABLES_DONE = True


def _bcast_cols(ap2d, col0, nblk, inner):
    """[128, nblk, inner] view of columns col0..col0+nblk of a [128, n] tile,
    each column replicated `inner` times along a 0-stride inner dim."""
    return bass.AP(
        tensor=ap2d.tensor,
        offset=ap2d.offset + col0,
        ap=[ap2d.ap[0], [1, nblk], [0, inner]],
    )


def _body(tc, aps):
    nc = tc.nc
    xT, wqT, wkT, wvT, woT, trig, tri, ones, neglam, y = aps

    xT_r = xT.rearrange("(k p) t -> p k t", p=128)      # [128, 16, 2048]
    wqT_r = wqT.rearrange("(k p) n -> p k n", p=128)    # [128, 16, 512]
    wkT_r = wkT.rearrange("(k p) n -> p k n", p=128)
    wvT_r = wvT.rearrange("(k p) n -> p k n", p=128)
    woT_r = woT.rearrange("(k p) n -> p k n", p=128)    # [128, 4, 2048]

    with ExitStack() as octx:
        # ---- persistent tiles (live across phases) ----
        persist = octx.enter_context(tc.tile_pool(name="persist", bufs=1))
        qkT_all = persist.tile([128, 8, T], BF16)    # seg h*4 + [q1,q2,k1,k2]
        v_all = persist.tile([128, NM, 512], BF16)   # [t(P) per m, e: h0|h1]
        trig_sb = persist.tile([128, 512], F32)      # cosE | sinE, block-major
        tri_sb = persist.tile([128, 128], BF16)
        ones_sb = persist.tile([128, 1], BF16)
        neglam_sb = persist.tile([128, 1], F32)
        ident_sb = persist.tile([128, 128], BF16)

        nc.sync.dma_start(out=trig_sb, in_=trig)
        nc.sync.dma_start(out=tri_sb, in_=tri)
        nc.sync.dma_start(out=ones_sb, in_=ones)
        nc.sync.dma_start(out=neglam_sb, in_=neglam)
        make_identity(nc, ident_sb)

        cosE = trig_sb[:, 0:256].rearrange("p (b i) -> p b i", b=4)
        sinE = trig_sb[:, 256:512].rearrange("p (b i) -> p b i", b=4)

        # segment mapping: qkT_all viewed [128, h, 4, T]; q -> [:, :, 0:2],
        # k -> [:, :, 2:4]; block order within a group is (h0b1,h0b2,h1b1,h1b2)
        qkT_v = qkT_all.rearrange("p (h f) t -> p h f t", h=2)

        # ================= Phase A: projections =================
        with ExitStack() as actx:
            wpool = actx.enter_context(tc.tile_pool(name="wpool", bufs=1))
            xpool = actx.enter_context(tc.tile_pool(name="xpool", bufs=1))
            psA = actx.enter_context(
                tc.tile_pool(name="psA", bufs=6, space="PSUM"))
            psT = actx.enter_context(
                tc.tile_pool(name="psT", bufs=2, space="PSUM"))
            sbA = actx.enter_context(tc.tile_pool(name="sbA", bufs=2))

            wq_sb = wpool.tile([128, NK, 512], BF16)
            wk_sb = wpool.tile([128, NK, 512], BF16)
            wv_sb = wpool.tile([128, NK, 512], BF16)
            x_sb = xpool.tile([128, NK, T], BF16)
            for kk in range(NK):
                nc.sync.dma_start(out=x_sb[:, kk, :], in_=xT_r[:, kk, :])
                nc.sync.dma_start(out=wq_sb[:, kk, :], in_=wqT_r[:, kk, :])
                nc.sync.dma_start(out=wk_sb[:, kk, :], in_=wkT_r[:, kk, :])
                nc.sync.dma_start(out=wv_sb[:, kk, :], in_=wvT_r[:, kk, :])

            for m in range(NM):
                mlo, mhi = m * 128, (m + 1) * 128
                q_ps = psA.tile([128, 512], F32, tag="acc")
                k_ps = psA.tile([128, 512], F32, tag="acc")
                v_ps = psA.tile([128, 512], F32, tag="acc")
                for kk in range(NK):
                    lhs = x_sb[:, kk, mlo:mhi]
                    st, sp = kk == 0, kk == NK - 1
                    nc.tensor.matmul(q_ps, lhs, wq_sb[:, kk, :], start=st, stop=sp)
                    nc.tensor.matmul(k_ps, lhs, wk_sb[:, kk, :], start=st, stop=sp)
                    nc.tensor.matmul(v_ps, lhs, wv_sb[:, kk, :], start=st, stop=sp)

                # vacate PSUM quickly: stage q/k to SBUF, v straight out
                qs = sbA.tile([128, 512], F32, tag="qs")
                nc.scalar.copy(qs, q_ps)
                ks = sbA.tile([128, 512], F32, tag="ks")
                nc.vector.tensor_copy(ks, k_ps)
                nc.scalar.copy(v_all[:, m, :], v_ps)

                # sumsq of the 8 blocks: stats cols 0-3 = q, 4-7 = k
                stats = sbA.tile([128, 8], F32, tag="stats")
                sq_scr = sbA.tile([128, 128], F32, tag="sqscr")
                for j in range(8):
                    src = qs if j < 4 else ks
                    off = (j % 4) * 128
                    nc.scalar.activation(
                        sq_scr, src[:, off:off + 128],
                        mybir.ActivationFunctionType.Square,
                        accum_out=stats[:, j:j + 1])
                # rstd = exp(-0.5*ln(mean+eps)) -- ln/exp/square share a table
                veps = sbA.tile([128, 8], F32, tag="veps")
                nc.vector.tensor_scalar(
                    veps, stats, 1.0 / HH, RMS_EPS,
                    mybir.AluOpType.mult, mybir.AluOpType.add)
                lnv = sbA.tile([128, 8], F32, tag="lnv")
                nc.scalar.activation(
                    lnv, veps, mybir.ActivationFunctionType.Ln)
                rstd8 = sbA.tile([128, 8], F32, tag="rstd8")
                nc.scalar.activation(
                    rstd8, lnv, mybir.ActivationFunctionType.Exp, scale=-0.5)

                # normalize + rotary, batched over all 4 blocks of q then k
                for g, (src, c0) in enumerate(((qs, 0), (ks, 4))):
                    qn = sbA.tile([128, 512], F32, tag="qn")
                    nc.vector.tensor_tensor(
                        qn.rearrange("p (b i) -> p b i", b=4), src.rearrange(
                            "p (b i) -> p b i", b=4),
                        _bcast_cols(rstd8, c0, 4, 128), mybir.AluOpType.mult)
                    qn3 = qn.rearrange("p (b i) -> p b i", b=4)
                    qn_lo, qn_hi = qn3[:, :, 0:64], qn3[:, :, 64:128]
                    t1 = sbA.tile([128, 256], F32, tag="t1")
                    t13 = t1.rearrange("p (b i) -> p b i", b=4)
                    t2 = sbA.tile([128, 256], F32, tag="t2")
                    t23 = t2.rearrange("p (b i) -> p b i", b=4)
                    qnr = sbA.tile([128, 512], BF16, tag="qnr")
                    qnr3 = qnr.rearrange("p (b i) -> p b i", b=4)
                    nc.vector.tensor_tensor(t13, qn_lo, cosE,
                                            mybir.AluOpType.mult)
                    nc.vector.tensor_tensor(t23, qn_hi, sinE,
                                            mybir.AluOpType.mult)
                    nc.vector.tensor_add(qnr3[:, :, 0:64], t13, t23)
                    nc.vector.tensor_tensor(t13, qn_hi, cosE,
                                            mybir.AluOpType.mult)
                    nc.vector.tensor_tensor(t23, qn_lo, sinE,
                                            mybir.AluOpType.mult)
                    nc.vector.tensor_sub(qnr3[:, :, 64:128], t13, t23)

                    tp = psT.tile([128, 512], BF16, tag="tp")
                    for bi in range(4):
                        nc.tensor.transpose(
                            tp[:, bi * 128:(bi + 1) * 128],
                            qnr[:, bi * 128:(bi + 1) * 128], ident_sb)
                    # dst: [128, h(2), br(2), 128] at segment group g
                    nc.vector.tensor_copy(
                        qkT_v[:, :, 2 * g:2 * g + 2, mlo:mhi],
                        tp.rearrange("p (h b i) -> p h b i", h=2, b=2))

        # ================= Phase B: attention =================
        bcpool = octx.enter_context(tc.tile_pool(name="bcpool", bufs=1))
        oT_all = bcpool.tile([128, 4, T], BF16)      # seg h*2 + etile
        wo_sb = bcpool.tile([128, 4, T], BF16)
        nc.sync.dma_start(out=wo_sb[:, 0, :], in_=woT_r[:, 0, :])
        nc.sync.dma_start(out=wo_sb[:, 1, :], in_=woT_r[:, 1, :])
        nc.sync.dma_start(out=wo_sb[:, 2, :], in_=woT_r[:, 2, :])
        nc.sync.dma_start(out=wo_sb[:, 3, :], in_=woT_r[:, 3, :])

        with ExitStack() as bctx:
            psS = bctx.enter_context(
                tc.tile_pool(name="psS", bufs=3, space="PSUM"))
            psO = bctx.enter_context(
                tc.tile_pool(name="psO", bufs=5, space="PSUM"))
            sbPT = bctx.enter_context(tc.tile_pool(name="sbPT", bufs=34))
            sbB = bctx.enter_context(tc.tile_pool(name="sbB", bufs=4))

            for h in range(2):
                qT = [qkT_v[:, h, 0, :], qkT_v[:, h, 1, :]]
                kT = [qkT_v[:, h, 2, :], qkT_v[:, h, 3, :]]
                for ch in range(NCH):
                    c0, c1 = ch * 512, ch * 512 + 512
                    ntk = 4 * ch + 4
                    # --- S + exp, both branches ---
                    br_pts = []
                    for br in range(2):
                        pts = []
                        for tkb in range(ntk):
                            n0 = max(c0, tkb * 128)
                            nN = c1 - n0
                            col0 = n0 - c0
                            s_ps = psS.tile([128, 512], F32, tag="s")
                            nc.tensor.matmul(
                                s_ps[:, :nN],
                                kT[br][:, tkb * 128:(tkb + 1) * 128],
                                qT[br][:, n0:c1], start=True, stop=True)
                            pt = sbPT.tile([128, 512], BF16, tag="pt")
                            nc.scalar.activation(
                                pt[:, :nN], s_ps[:, :nN],
                                mybir.ActivationFunctionType.Exp, scale=SCALE)
                            if tkb * 128 >= c0:   # diagonal block
                                nc.vector.tensor_mul(
                                    pt[:, :128], pt[:, :128], tri_sb)
                            pts.append((tkb, pt, nN, col0))
                        br_pts.append(pts)
                    # --- den (ones-matmuls) + PV; PE: den0, PV0, den1, PV1 ---
                    o_ps = {}
                    den_ps = {}
                    for br in range(2):
                        dp = psS.tile([1, 512], F32, tag="s", name=f"den{br}")
                        for i, (tkb, pt, nN, col0) in enumerate(br_pts[br]):
                            nc.tensor.matmul(
                                dp[:, col0:], ones_sb, pt[:, :nN],
                                start=(i == 0), stop=(i == ntk - 1))
                        den_ps[br] = dp
                        o_ps[br] = [psO.tile([128, 512], F32, tag="o",
                                             name=f"o_ps{br}{e}")
                                    for e in range(2)]
                        for e in range(2):
                            ecol = h * 256 + e * 128
                            for i, (tkb, pt, nN, col0) in enumerate(br_pts[br]):
                                nc.tensor.matmul(
                                    o_ps[br][e][:, col0:],
                                    v_all[:, tkb, ecol:ecol + 128],
                                    pt[:, :nN],
                                    start=(i == 0), stop=(i == ntk - 1))
                    # --- 1/den (fast approx), broadcast, combine ---
                    invb = {}
                    for br in range(2):
                        inv = sbB.tile([1, 512], F32, tag="inv",
                                       name=f"inv{br}")
                        nc.vector.reciprocal_approx_fast(inv, den_ps[br])
                        ib = sbB.tile([128, 512], F32, tag="invb",
                                      name=f"invb{br}")
                        nc.gpsimd.partition_broadcast(ib, inv)
                        invb[br] = ib
                    for e in range(2):
                        o1n = sbB.tile([128, 512], F32, tag="o1n")
                        o2n = sbB.tile([128, 512], F32, tag="o2n")
                        nc.vector.tensor_mul(o1n, o_ps[0][e], invb[0])
                        nc.vector.tensor_mul(o2n, o_ps[1][e], invb[1])
                        nc.vector.scalar_tensor_tensor(
                            oT_all[:, h * 2 + e, c0:c1], o2n, neglam_sb, o1n,
                            mybir.AluOpType.mult, mybir.AluOpType.add)

        # ================= Phase C: out projection =================
        with ExitStack() as cctx:
            psY = cctx.enter_context(
                tc.tile_pool(name="psY", bufs=8, space="PSUM"))
            sbY = cctx.enter_context(tc.tile_pool(name="sbY", bufs=2))
            for m in range(NM):
                mlo, mhi = m * 128, (m + 1) * 128
                y_ps = [psY.tile([128, 512], F32, tag="y", name=f"y_ps{cc}")
                        for cc in range(4)]
                for kk in range(4):
                    lhs = oT_all[:, kk, mlo:mhi]
                    for cc in range(4):
                        nc.tensor.matmul(
                            y_ps[cc], lhs, wo_sb[:, kk, cc * 512:(cc + 1) * 512],
                            start=(kk == 0), stop=(kk == 3))
                ystage = sbY.tile([128, T], F32, tag="ystage")
                for cc in range(4):
                    if cc % 2 == 0:
                        nc.vector.tensor_copy(
                            ystage[:, cc * 512:(cc + 1) * 512], y_ps[cc])
                    else:
                        nc.scalar.copy(
                            ystage[:, cc * 512:(cc + 1) * 512], y_ps[cc])
                nc.sync.dma_start(out=y[mlo:mhi, :], in_=ystage)


def build_nc():
    _setup_act_tables()
    nc = bacc.Bacc("TRN2", target_bir_lowering=False, debug=False,
                   num_devices=8)
    xT = nc.dram_tensor("xT", [C, T], BF16, kind="ExternalInput").ap()
    wqT = nc.dram_tensor("wqT", [C, 512], BF16, kind="ExternalInput").ap()
    wkT = nc.dram_tensor("wkT", [C, 512], BF16, kind="ExternalInput").ap()
    wvT = nc.dram_tensor("wvT", [C, 512], BF16, kind="ExternalInput").ap()
    woT = nc.dram_tensor("woT", [512, C], BF16, kind="ExternalInput").ap()
    trig = nc.dram_tensor("trig", [128, 512], F32, kind="ExternalInput").ap()
    tri = nc.dram_tensor("tri", [128, 128], BF16, kind="ExternalInput").ap()
    ones = nc.dram_tensor("ones", [128, 1], BF16, kind="ExternalInput").ap()
    neglam = nc.dram_tensor("neglam", [128, 1], F32,
                            kind="ExternalInput").ap()
    y = nc.dram_tensor("y", [T, C], F32, kind="ExternalOutput").ap()
    with tile.TileContext(nc) as tc:
        _body(tc, (xT, wqT, wkT, wvT, woT, trig, tri, ones, neglam, y))
    nc.compile()
    return nc


def _host_prep(x, wq, wk, wv, wo, lq1, lk1, lq2, lk2):
    x = np.asarray(x, np.float32)
    wq, wk, wv, wo = (np.asarray(w, np.float32) for w in (wq, wk, wv, wo))
    lam = float(np.exp(np.sum(np.asarray(lq1, np.float32) *
                              np.asarray(lk1, np.float32))) -
                np.exp(np.sum(np.asarray(lq2, np.float32) *
                              np.asarray(lk2, np.float32))) + LAMBDA_INIT)

    d = HH
    inv_freq = (1.0 / 10000.0) ** (np.arange(0, d, 2, dtype=np.float32) / d)
    freqs = np.outer(np.arange(NH, dtype=np.float32), inv_freq)
    cos, sin = np.cos(freqs), np.sin(freqs)

    tri = np.triu(np.ones((128, 128), np.float32)).astype(NPBF16)
    ones = np.ones((128, 1), np.float32).astype(NPBF16)
    neglam = np.full((128, 1), -lam, np.float32)

    in_maps = []
    for core in range(8):
        b = core // 4
        hp = core % 4
        h0, h1 = 2 * hp, 2 * hp + 1
        rows = np.r_[h0 * 256:(h0 + 1) * 256, h1 * 256:(h1 + 1) * 256]
        # block order (h0, h0, h1, h1): cosE | sinE, each [128, 4*64]
        cosE = np.concatenate(
            [np.tile(cos[hh][None, :], (128, 1)) for hh in (h0, h0, h1, h1)],
            axis=1)
        sinE = np.concatenate(
            [np.tile(sin[hh][None, :], (128, 1)) for hh in (h0, h0, h1, h1)],
            axis=1)
        trig_t = np.concatenate([cosE, sinE], axis=1).astype(np.float32)
        in_maps.append({
            "xT": np.ascontiguousarray(x[b].T).astype(NPBF16),
            "wqT": np.ascontiguousarray(wq[rows, :].T).astype(NPBF16),
            "wkT": np.ascontiguousarray(wk[rows, :].T).astype(NPBF16),
            "wvT": np.ascontiguousarray(wv[rows, :].T).astype(NPBF16),
            "woT": np.ascontiguousarray(
                (wo[:, rows].T * (1.0 - LAMBDA_INIT))).astype(NPBF16),
            "trig": trig_t,
            "tri": tri,
            "ones": ones,
            "neglam": neglam,
        })
    return in_maps


def kernel(x, wq, wk, wv, wo, lq1, lk1, lq2, lk2, _results_out=None,
           _trace=False):
    in_maps = _host_prep(x, wq, wk, wv, wo, lq1, lk1, lq2, lk2)
    nc = build_nc()
    res = bass_utils.run_bass_kernel_spmd(nc, in_maps,
                                          core_ids=list(range(8)),
                                          trace=_trace)
    if _results_out is not None:
        _results_out.append(res)
    out = np.zeros((B, T, C), np.float32)
    for core in range(8):
        out[core // 4] += res.results[core]["y"]
    return out
